# revision 5
# baseline (speedup 1.0000x reference)
"""Trainium2 Bass kernel for nn_AutoregressiveDecoder (gnn_message_passing), v2.

Math restructuring: with Ahat = max(adj, I),
  CS[i,u] = sum_{v<i} Ahat[v,u], deg_j = rsqrt(max(CS[i_j],1)) (masked v<i_j),
  X_j[h,u] = sum_v Z1[v,h] deg_j[v] Ahat[v,u]     (Z1 = z @ W1[:128])
  t_j[u]   = sum_h relu(X_j)[h,u] rc_j[h],        rc_j = W2 @ q_j
  q_j      = relu(z_{i_j} @ W1[:128] + W1[128]) @ W2
  supp row i_j = 0.5*tanh(deg_j ∘ (Ahat @ (deg_j^2 ∘ t_j))) + 0.5*tanh(q.q) e_j
  out = x + supp + supp^T,  x = 0.5 z z^T.
Row-parallel across 8 cores, core c handles rows i = c, c+8, ..., c+248.

Engine/schedule plan (driven by timeline-sim iterations):
  - PE is strictly in-order: stage-B matmuls are emitted DEPTH units after
    their stage-A matmuls (software pipeline) so PE never stalls on the
    relu-evacuation of the row it just computed.
  - Stage B for rows 0..27: one fp8 DoubleRow matmul per row with a
    zero-padded [128,2,32] weight (col j = rc_j) accumulating t_j into a
    persistent psum tile spT[j, u]; spT closes at row 27 and its
    copy/transpose tail overlaps rows 28..31, which use direct per-row
    matvecs (F as weight) into a [u, j] psum tile instead.
  - relu evacuation (psum fp32 -> sbuf fp8) is the throughput limiter:
    rows 0..15 are evacuated in pairs sharing one psum bank (halves the
    per-op PSUM access penalty), split DVE/ACT by row for balance.
  - deg via quake-rsqrt on DVE; CS in gathered [u, j] layout from an fp8
    copy of Ahat+mask (exact for 0/1 data, smallest possible first DMA).
  - Only one activation-table set (Tanh/Relu/Copy), warmed by a dummy tanh.
  - x = 0.5 z z^T as bf16 hi/lo Karatsuba (error ~2e-5).
"""

import numpy as np

N = 256
DIN = 128
H1 = 256
H2 = 128
NCORES = 8
NPC = N // NCORES   # 32 rows per core
NSPT = 24           # rows using the spT accumulation path; rest are direct

# blob8 (fp8e4) column layout: CS-chain inputs + masks
_8AH = 0       # ahat two v-blocks        [128, 512]
_8MC = 512     # mc two v-blocks          [128, 64]
_8COLS = 576

# blobb (bf16) column layout (roughly in order of first use)
_ZTH = 0       # z^T hi                   [128, 256]
_ZRM = 256     # z row-major two i-blocks [128, 256]
_W1A = 512     # W1[0:128]                [128, 256]
_OCB = 768     # one-hot bf16             [128, 64]
_W2B = 832     # W2 row-blocks            [128, 256]
_W2T = 1088    # W2^T                     [128, 256]
_AH = 1344     # ahat two v-blocks        [128, 512]
_ZTL = 1856    # z^T lo                   [128, 256]
_IDB = 2112    # identity                 [128, 128]
_BCOLS = 2240

# blobf (fp32) column layout
_OCF = 0       # one-hot fp32             [128, 64]
_W1B = 64      # W1[128] as [128, 2]      [128, 2]
_FCOLS = 66

_QMAGIC = 0x5F3759DF

# rows >= EVAC_SPLIT evacuate on ACT; earlier rows on DVE (pairs below 16)
EVAC_SPLIT = 22
# software-pipeline depth (units = row-pairs for j<16, rows for j>=16)
DEPTH = 2

_PROGRAM = None
LAST_RESULTS = None
TRACE = False
TRACE_KW = {}


def _build_program():
    import concourse.bacc as bacc
    import concourse.mybir as mybir
    from concourse import tile

    F32 = mybir.dt.float32
    BF16 = mybir.dt.bfloat16
    FP8 = mybir.dt.float8e4
    I32 = mybir.dt.int32
    AF = mybir.ActivationFunctionType
    ALU = mybir.AluOpType
    DR = mybir.MatmulPerfMode.DoubleRow

    nc = bacc.Bacc()

    blob8_d = nc.dram_tensor("blob8", [128, _8COLS], FP8, kind="ExternalInput")
    blobb_d = nc.dram_tensor("blobb", [128, _BCOLS], BF16, kind="ExternalInput")
    blobf_d = nc.dram_tensor("blobf", [128, _FCOLS], F32, kind="ExternalInput")
    outc_d = nc.dram_tensor("outc", [N, NPC], F32, kind="ExternalOutput")
    x_d = nc.dram_tensor("xout", [N, N], F32, kind="ExternalOutput")
    tq_d = nc.dram_tensor("tqout", [1, NPC], F32, kind="ExternalOutput")

    with tile.TileContext(nc) as tc, tc.tile_pool(name="persist", bufs=1) as P:
        blob8 = P.tile([128, _8COLS], FP8, tag="blob8", name="blob8")
        blobb = P.tile([128, _BCOLS], BF16, tag="blobb", name="blobb")
        blobf = P.tile([128, _FCOLS], F32, tag="blobf", name="blobf")
        nc.sync.dma_start(blob8[:], blob8_d[:])          # critical path, first
        nc.scalar.dma_start(blobb[:, 0:_W2B], blobb_d[:, 0:_W2B])
        nc.scalar.dma_start(blobb[:, _W2B:_AH], blobb_d[:, _W2B:_AH])
        nc.sync.dma_start(blobb[:, _AH:_BCOLS], blobb_d[:, _AH:_BCOLS])
        nc.scalar.dma_start(blobf[:], blobf_d[:])

        ah8 = [blob8[:, _8AH + b * 256:_8AH + (b + 1) * 256] for b in range(2)]
        mc8 = [blob8[:, _8MC + b * 32:_8MC + (b + 1) * 32] for b in range(2)]

        def ahb(vb, ub=None):
            if ub is None:
                return blobb[:, _AH + vb * 256:_AH + (vb + 1) * 256]
            return blobb[:, _AH + vb * 256 + ub * 128:_AH + vb * 256 + (ub + 1) * 128]

        zthi = blobb[:, _ZTH:_ZTH + 256]
        ztlo = blobb[:, _ZTL:_ZTL + 256]
        zrm = [blobb[:, _ZRM + b * 128:_ZRM + (b + 1) * 128] for b in range(2)]
        w1a = blobb[:, _W1A:_W1A + 256]
        w2b = [blobb[:, _W2B + b * 128:_W2B + (b + 1) * 128] for b in range(2)]
        w2t = blobb[:, _W2T:_W2T + 256]
        identb = blobb[:, _IDB:_IDB + 128]
        ocb = [blobb[:, _OCB + b * 32:_OCB + (b + 1) * 32] for b in range(2)]
        ocf = [blobf[:, _OCF + b * 32:_OCF + (b + 1) * 32] for b in range(2)]
        w1bcol = [blobf[:, _W1B + b:_W1B + b + 1] for b in range(2)]

        # ---- tiny constants; dummy tanh loads the one activation table ----
        zed = P.tile([128, 1], F32, tag="zed", name="zed")
        nc.vector.memset(zed[:], 0.0)
        thd = P.tile([128, 1], F32, tag="thd", name="thd")
        nc.scalar.activation(thd[:], zed[:], AF.Tanh)
        ones_col = P.tile([128, 1], BF16, tag="ones_col", name="ones_col")
        nc.vector.memset(ones_col[:], 1.0)
        kmagic = P.tile([128, 2, 32], I32, tag="kmagic", name="kmagic")
        nc.vector.memset(kmagic[:], _QMAGIC)
        # zero-padded stage-B weights: rcpad[:, hb, j, :] has col j = rc_j[hb]
        rcpad = P.tile([128, 2, 32, 32], FP8, tag="rcpad", name="rcpad")
        nc.gpsimd.memset(rcpad[:].bitcast(I32), 0)

        # persistent sbuf intermediates
        cmax = P.tile([128, 2, 32], F32, tag="cmax", name="cmax")
        shi = P.tile([128, 2, 32], I32, tag="shi", name="shi")
        yq = P.tile([128, 2, 32], F32, tag="yq", name="yq")
        qa = P.tile([128, 2, 32], F32, tag="qa", name="qa")
        qb = P.tile([128, 2, 32], F32, tag="qb", name="qb")
        yq2 = P.tile([128, 2, 32], F32, tag="yq2", name="yq2")
        yq3 = P.tile([128, 2, 32], F32, tag="yq3", name="yq3")
        degc = P.tile([128, 2, 32], F32, tag="degc", name="degc")
        ddct = P.tile([128, 2, 32], F32, tag="ddct", name="ddct")
        z1b = P.tile([128, 2, 256], BF16, tag="z1b", name="z1b")
        zcb = P.tile([128, 32], BF16, tag="zcb", name="zcb")
        rbcb = P.tile([128, 2, 32], BF16, tag="rbcb", name="rbcb")
        qcb = P.tile([128, 32], BF16, tag="qcb", name="qcb")
        rcc = P.tile([128, 2, 32], BF16, tag="rcc", name="rcc")
        sqc = P.tile([128, 32], BF16, tag="sqc", name="sqc")
        qqsb = P.tile([1, 32], F32, tag="qqsb", name="qqsb")
        tqrow = P.tile([1, 32], F32, tag="tqrow", name="tqrow")
        xsb = P.tile([128, 2, 256], F32, tag="xsb", name="xsb")

        with tc.tile_pool(name="pre_big", bufs=2, space="PSUM") as PB, \
             tc.tile_pool(name="pre_small", bufs=3, space="PSUM") as PS:
            # ---- CS chain (critical path): CS[i_j, u] in [u, j] layout ----
            csps = PS.tile([128, 2, 32], F32, tag="pps", name="csps")
            for ub in range(2):
                for vb in range(2):
                    nc.tensor.matmul(csps[:, ub, :],
                                     ah8[vb][:, ub * 128:(ub + 1) * 128], mc8[vb],
                                     start=(vb == 0), stop=(vb == 1))
            nc.vector.tensor_scalar_max(cmax[:], csps[:], 1.0)
            # quake rsqrt: y0 = bits(K - (bits(c) >> 1)); 2 Newton iters
            nc.vector.tensor_single_scalar(shi[:], cmax[:].bitcast(I32), 1,
                                           ALU.logical_shift_right)
            nc.vector.tensor_sub(yq[:].bitcast(I32), kmagic[:], shi[:])
            nc.vector.tensor_mul(qa[:], yq[:], yq[:])
            nc.vector.scalar_tensor_tensor(qb[:], qa[:], -0.5, cmax[:],
                                           ALU.mult, ALU.mult)
            nc.vector.scalar_tensor_tensor(yq3[:], qb[:], 1.5, yq[:],
                                           ALU.add, ALU.mult)
            for b in range(2):
                nc.vector.tensor_mul(degc[:, b, :], yq3[:, b, :], mc8[b])
            nc.vector.tensor_mul(ddct[:], degc[:], yq3[:])

            # ---- Z1 = z @ W1a (stage-A weights) ----
            z1ps = PB.tile([128, 2, 256], F32, tag="ppb", name="z1ps")
            for b in range(2):
                nc.tensor.matmul(z1ps[:, b, :], zthi[:, b * 128:(b + 1) * 128],
                                 w1a, start=True, stop=True)
            nc.scalar.activation(z1b[:], z1ps[:], AF.Copy)


        # ---------------- per-row loop (software-pipelined) ----------------
        spTb = P.tile([32, 256], BF16, tag="spTb", name="spTb")
        sprime = P.tile([128, 2, 32], BF16, tag="sprime", name="sprime")
        wm = P.tile([128, 2, 32], F32, tag="wm", name="wm")
        th = P.tile([128, 2, 32], F32, tag="th", name="th")

        with tc.tile_pool(name="spt", bufs=1, space="PSUM") as SPP, \
             tc.tile_pool(name="tail_ps", bufs=1, space="PSUM") as TP, \
             tc.tile_pool(name="loop_ps2", bufs=2, space="PSUM") as LP2, \
             tc.tile_pool(name="loop_psb", bufs=2, space="PSUM") as LPB, \
             tc.tile_pool(name="loop_s", bufs=16) as LS, \
             tc.tile_pool(name="loop_f", bufs=8) as LF:
            spT = SPP.tile([128, 512], F32, tag="spt", name="spT")   # own bank
            spdir = TP.tile([128, 2, 8], F32, tag="spd", name="spdir", bufs=1)

            def qstage_zc():
                zcps = TP.tile([128, 2, 32], F32, tag="spd", name="zcps", bufs=1)
                for ib in range(2):
                    nc.tensor.matmul(zcps[:, 0, :], zrm[ib], ocb[ib],
                                     start=(ib == 0), stop=(ib == 1))
                nc.scalar.activation(zcb[:], zcps[:, 0, :], AF.Copy)

            def qstage_rbc():
                rbcps = TP.tile([128, 2, 32], F32, tag="spd", name="rbcps", bufs=1)
                for hb in range(2):
                    nc.tensor.matmul(rbcps[:, hb, :],
                                     w1a[:, hb * 128:(hb + 1) * 128],
                                     zcb[:], start=True, stop=True)
                for hb in range(2):
                    nc.scalar.activation(rbcb[:, hb, :], rbcps[:, hb, :], AF.Relu,
                                         bias=w1bcol[hb])

            def qstage_qc():
                qcps = TP.tile([128, 2, 32], F32, tag="spd", name="qcps", bufs=1)
                for hb in range(2):
                    nc.tensor.matmul(qcps[:, 0, :], w2b[hb], rbcb[:, hb, :],
                                     start=(hb == 0), stop=(hb == 1))
                nc.scalar.activation(qcb[:], qcps[:, 0, :], AF.Copy)

            def qstage_rc():
                rcps = TP.tile([128, 2, 32], F32, tag="spd", name="rcps", bufs=1)
                for hb in range(2):
                    nc.tensor.matmul(rcps[:, hb, :],
                                     w2t[:, hb * 128:(hb + 1) * 128],
                                     qcb[:], start=True, stop=True)
                nc.scalar.activation(rcc[:], rcps[:], AF.Copy)
                for hb in range(2):
                    dst = rcpad[:, hb].rearrange("p a b -> p (a b)")[:, ::33]
                    nc.vector.tensor_copy(dst, rcps[:, hb, :])

            def qstage_qq():
                nc.vector.tensor_mul(sqc[:], qcb[:], qcb[:])
                qqps = TP.tile([128, 2, 32], F32, tag="spd", name="qqps", bufs=1)
                nc.tensor.matmul(qqps[0:1, 0, :], ones_col[:], sqc[:],
                                 start=True, stop=True)
                nc.vector.tensor_copy(qqsb[:], qqps[0:1, 0, :])
                nc.scalar.activation(tqrow[:], qqsb[:], AF.Tanh)
                nc.sync.dma_start(tq_d[:], tqrow[:])

            def qstage_x():
                xps = LPB.tile([128, 2, 2, 256], F32, tag="psx", name="xps")
                for b in range(2):
                    hi_b = zthi[:, b * 128:(b + 1) * 128]
                    lo_b = ztlo[:, b * 128:(b + 1) * 128]
                    nc.tensor.matmul(xps[:, 0, b, :], hi_b, zthi,
                                     start=True, stop=False)
                    nc.tensor.matmul(xps[:, 0, b, :], hi_b, ztlo,
                                     start=False, stop=False)
                    nc.tensor.matmul(xps[:, 0, b, :], lo_b, zthi,
                                     start=False, stop=True)
                nc.scalar.activation(xsb[:], xps[:, 0], AF.Copy, scale=0.5)
                for b in range(2):
                    nc.sync.dma_start(x_d[b * 128:(b + 1) * 128, :], xsb[:, b, :])

            qstages = [qstage_zc, qstage_rbc, qstage_qc, qstage_rc, qstage_qq,
                       qstage_x]
            # trps/wps recycle spT's bank: each is first used only after the
            # previous one's last read (copy -> transpose -> fold)
            trps = SPP.tile([128, 2, 32], BF16, tag="spt", name="trps", bufs=1)
            wps = SPP.tile([128, 2, 32], F32, tag="spt", name="wps", bufs=1)

            def s_scale(j, vb, engine):
                mj = 8 * (j + 1)
                sz = min(mj, 128) if vb == 0 else mj - 128
                s = LS.tile([128, 256], BF16, tag=f"s{vb}", name=f"s{vb}_{j}")
                engine.tensor_scalar_mul(s[0:sz, 0:mj], ahb(vb)[0:sz, 0:mj],
                                         degc[0:sz, vb, j:j + 1])
                return s, sz

            def stage_a(j, psx_slice):
                mj = 8 * (j + 1)
                nvb = 1 if mj <= 128 else 2
                svt = []
                for vb in range(nvb):
                    pool = (vb == 1 and j < 28) or (vb == 0 and j < 8)
                    svt.append(s_scale(j, vb, nc.gpsimd if pool else nc.vector))
                for hb in range(2):
                    for vb in range(nvb):
                        s, sz = svt[vb]
                        nc.tensor.matmul(
                            psx_slice[:, hb, 0:mj],
                            z1b[0:sz, vb, hb * 128:(hb + 1) * 128],
                            s[0:sz, 0:mj],
                            start=(vb == 0), stop=(vb == nvb - 1))

            def stage_b(j, ft):
                # ft: [128, 2, mj] fp8 view (interleaved K-pair layout for DR)
                mj = 8 * (j + 1)
                if j < NSPT:
                    nc.tensor.matmul(spT[0:32, 0:mj], rcpad[:, :, j, :], ft,
                                     perf_mode=DR,
                                     start=(j == 0), stop=(j == NSPT - 1))
                else:
                    for vb in range(2):
                        sz = min(mj, 128) if vb == 0 else mj - 128
                        for hb in range(2):
                            nc.tensor.matmul(
                                spdir[0:sz, vb, j - NSPT:j - NSPT + 1],
                                ft[:, hb, vb * 128:vb * 128 + sz],
                                rcc[:, hb, j:j + 1],
                                start=(hb == 0), stop=(hb == 1))

            def tail_a():
                # spT closed: copy/transpose/scale the first NSPT columns
                nc.vector.tensor_copy(spTb[:], spT[0:32, 0:256])
                for ub in range(2):
                    nc.tensor.transpose(trps[:, ub, :],
                                        spTb[:, ub * 128:(ub + 1) * 128],
                                        identb[0:32, 0:32])
                nc.vector.tensor_mul(sprime[:], trps[:], ddct[:])

            # pipeline: emit stage_b DEPTH units behind stage_a (ascending
            # rows; 0..23 accumulate into spT, 24..31 use direct matvecs so
            # the spT transpose tail overlaps the last rows)
            pend = []   # (j, ft_view)

            def flush(limit):
                while len(pend) > limit:
                    jj, ftv = pend.pop(0)
                    stage_b(jj, ftv)
                    if jj == NSPT - 1:
                        tail_a()

            for jp in range(8):          # rows 0..15 in pairs (1 bank)
                j0, j1 = 2 * jp, 2 * jp + 1
                mj1 = 8 * (j1 + 1)
                psX2 = LP2.tile([128, 2, 2, 128], F32, tag="psx2",
                                name=f"psx2_{jp}")
                stage_a(j0, psX2[:, 0])
                stage_a(j1, psX2[:, 1])
                ft2 = LF.tile([128, 2, 2, 128], FP8, tag="ft2", name=f"ftp{jp}",
                              bufs=8)
                if jp >= 2:
                    nc.scalar.activation(ft2[:, :, :, 0:mj1],
                                         psX2[:, :, :, 0:mj1], AF.Relu)
                else:
                    nc.vector.tensor_scalar_max(ft2[:, :, :, 0:mj1],
                                                psX2[:, :, :, 0:mj1], 0.0)
                pend.append((j0, ft2[:, 0, :, 0:8 * (j0 + 1)]))
                pend.append((j1, ft2[:, 1, :, 0:mj1]))
                if jp < len(qstages):
                    qstages[jp]()

            for jp in range(8, 16):      # rows 16..31 in pairs (2 banks)
                j0, j1 = 2 * jp, 2 * jp + 1
                mj1 = 8 * (j1 + 1)
                psXB = LPB.tile([128, 2, 2, 256], F32, tag="psx", name=f"psxb{jp}")
                stage_a(j0, psXB[:, 0])
                stage_a(j1, psXB[:, 1])
                ftb = LF.tile([128, 2, 2, 256], FP8, tag="ftb", name=f"ftb{jp}")
                if jp == 15:
                    nc.scalar.activation(ftb[:, 0, :, 0:8 * (j0 + 1)],
                                         psXB[:, 0, :, 0:8 * (j0 + 1)], AF.Relu)
                    nc.vector.tensor_scalar_max(ftb[:, 1, :, 0:mj1],
                                                psXB[:, 1, :, 0:mj1], 0.0)
                elif jp in (8, 9, 10, 13):
                    nc.vector.tensor_scalar_max(ftb[:, :, :, 0:mj1],
                                                psXB[:, :, :, 0:mj1], 0.0)
                else:
                    nc.scalar.activation(ftb[:, :, :, 0:mj1],
                                         psXB[:, :, :, 0:mj1], AF.Relu)
                pend.append((j0, ftb[:, 0, :, 0:8 * (j0 + 1)]))
                pend.append((j1, ftb[:, 1, :, 0:mj1]))
                flush(max(DEPTH, 14 - 2 * (jp - 8)))
            flush(0)

            # ---- tail B, part 1: columns 0:NSPT (spT rows), ships early ----
            for ub in range(2):
                for vb in range(2):
                    nc.tensor.matmul(wps[:, ub, 0:NSPT], ahb(vb, ub),
                                     sprime[:, vb, 0:NSPT],
                                     start=(vb == 0), stop=(vb == 1))
            nc.vector.tensor_mul(wm[:, :, 0:NSPT], wps[:, :, 0:NSPT],
                                 degc[:, :, 0:NSPT])
            nc.scalar.activation(th[:, :, 0:NSPT], wm[:, :, 0:NSPT], AF.Tanh)
            nc.scalar.dma_start(outc_d[0:128, 0:NSPT], th[:, 0, 0:NSPT])
            nc.sync.dma_start(outc_d[128:256, 0:NSPT], th[:, 1, 0:NSPT])

            # ---- tail B, part 2: columns NSPT:NPC (direct rows) ----
            ND = NPC - NSPT
            nc.vector.tensor_mul(sprime[:, :, NSPT:NPC], spdir[:],
                                 ddct[:, :, NSPT:NPC])
            wps2 = TP.tile([128, 2, 8], F32, tag="spd", name="wps2", bufs=1)
            for ub in range(2):
                for vb in range(2):
                    nc.tensor.matmul(wps2[:, ub, :], ahb(vb, ub),
                                     sprime[:, vb, NSPT:NPC],
                                     start=(vb == 0), stop=(vb == 1))
            nc.vector.tensor_mul(wm[:, :, NSPT:NPC], wps2[:], degc[:, :, NSPT:NPC])
            nc.scalar.activation(th[:, :, NSPT:NPC], wm[:, :, NSPT:NPC], AF.Tanh)
            nc.scalar.dma_start(outc_d[0:128, NSPT:NPC], th[:, 0, NSPT:NPC])
            nc.sync.dma_start(outc_d[128:256, NSPT:NPC], th[:, 1, NSPT:NPC])

    nc.finalize()
    return nc


def _get_program():
    global _PROGRAM
    if _PROGRAM is None:
        _PROGRAM = _build_program()
    return _PROGRAM


def _pack_inputs(z, adj, W1, W2):
    import ml_dtypes
    bf = ml_dtypes.bfloat16
    f8 = ml_dtypes.float8_e4m3fn

    idx = np.arange(N)
    ahat = np.maximum(adj, np.eye(N, dtype=np.float32))
    zt = z.T.astype(np.float32)                     # [128, 256]
    zthi = zt.astype(bf)
    ztlo = (zt - zthi.astype(np.float32)).astype(bf)

    base_8 = np.zeros((128, _8COLS), f8)
    base_8[:, _8AH:_8AH + 256] = ahat[0:128].astype(f8)
    base_8[:, _8AH + 256:_8AH + 512] = ahat[128:256].astype(f8)

    base_b = np.zeros((128, _BCOLS), bf)
    base_b[:, _AH:_AH + 256] = ahat[0:128].astype(bf)
    base_b[:, _AH + 256:_AH + 512] = ahat[128:256].astype(bf)
    base_b[:, _ZTH:_ZTH + 256] = zthi
    base_b[:, _ZTL:_ZTL + 256] = ztlo
    base_b[:, _ZRM:_ZRM + 128] = z[0:128].astype(bf)
    base_b[:, _ZRM + 128:_ZRM + 256] = z[128:256].astype(bf)
    base_b[:, _W1A:_W1A + 256] = W1[0:128].astype(bf)
    base_b[:, _W2B:_W2B + 128] = W2[0:128].astype(bf)
    base_b[:, _W2B + 128:_W2B + 256] = W2[128:256].astype(bf)
    base_b[:, _W2T:_W2T + 256] = W2.T.astype(bf)
    base_b[:, _IDB:_IDB + 128] = np.eye(128, dtype=np.float32).astype(bf)

    base_f = np.zeros((128, _FCOLS), np.float32)
    base_f[:, _W1B] = W1[128, 0:128]
    base_f[:, _W1B + 1] = W1[128, 128:256]

    in_maps = []
    for c in range(NCORES):
        ii = np.arange(c, N, NCORES)
        OC = np.zeros((N, NPC), np.float32)
        OC[ii, np.arange(NPC)] = 1.0
        MC = (idx[:, None] < ii[None, :]).astype(np.float32)
        b8 = base_8.copy()
        b8[:, _8MC:_8MC + 32] = MC[0:128].astype(f8)
        b8[:, _8MC + 32:_8MC + 64] = MC[128:256].astype(f8)
        bb = base_b.copy()
        bb[:, _OCB:_OCB + 32] = OC[0:128].astype(bf)
        bb[:, _OCB + 32:_OCB + 64] = OC[128:256].astype(bf)
        bfl = base_f.copy()
        bfl[:, _OCF:_OCF + 32] = OC[0:128]
        bfl[:, _OCF + 32:_OCF + 64] = OC[128:256]
        in_maps.append({"blob8": b8, "blobb": bb, "blobf": bfl})
    return in_maps


def kernel(z, adj, W1, W2):
    global LAST_RESULTS
    from concourse.bass_utils import run_bass_kernel_spmd

    z = np.ascontiguousarray(np.asarray(z, np.float32))
    adj = np.ascontiguousarray(np.asarray(adj, np.float32))
    W1 = np.ascontiguousarray(np.asarray(W1, np.float32))
    W2 = np.ascontiguousarray(np.asarray(W2, np.float32))

    nc = _get_program()
    in_maps = _pack_inputs(z, adj, W1, W2)
    res = run_bass_kernel_spmd(nc, in_maps, list(range(NCORES)),
                               trace=TRACE, **TRACE_KW)
    LAST_RESULTS = res
    supp = np.zeros((N, N), np.float32)
    for c in range(NCORES):
        ii = np.arange(c, N, NCORES)
        supp[ii, :] = 0.5 * res.results[c]["outc"].T
        supp[ii, ii] += 0.5 * res.results[c]["tqout"][0]
    x = res.results[0]["xout"]
    return (x + supp + supp.T).astype(np.float32)


# revision 6
# speedup vs baseline: 1.0023x; 1.0023x over previous
"""Trainium2 Bass kernel for nn_AutoregressiveDecoder (gnn_message_passing), v2.

Math restructuring: with Ahat = max(adj, I),
  CS[i,u] = sum_{v<i} Ahat[v,u], deg_j = rsqrt(max(CS[i_j],1)) (masked v<i_j),
  X_j[h,u] = sum_v Z1[v,h] deg_j[v] Ahat[v,u]     (Z1 = z @ W1[:128])
  t_j[u]   = sum_h relu(X_j)[h,u] rc_j[h],        rc_j = W2 @ q_j
  q_j      = relu(z_{i_j} @ W1[:128] + W1[128]) @ W2
  supp row i_j = 0.5*tanh(deg_j ∘ (Ahat @ (deg_j^2 ∘ t_j))) + 0.5*tanh(q.q) e_j
  out = x + supp + supp^T,  x = 0.5 z z^T.
Row-parallel across 8 cores, core c handles rows i = c, c+8, ..., c+248.

Engine/schedule plan (driven by timeline-sim iterations):
  - PE is strictly in-order: stage-B matmuls are emitted DEPTH units after
    their stage-A matmuls (software pipeline) so PE never stalls on the
    relu-evacuation of the row it just computed.
  - Stage B for rows 0..27: one fp8 DoubleRow matmul per row with a
    zero-padded [128,2,32] weight (col j = rc_j) accumulating t_j into a
    persistent psum tile spT[j, u]; spT closes at row 27 and its
    copy/transpose tail overlaps rows 28..31, which use direct per-row
    matvecs (F as weight) into a [u, j] psum tile instead.
  - relu evacuation (psum fp32 -> sbuf fp8) is the throughput limiter:
    rows 0..15 are evacuated in pairs sharing one psum bank (halves the
    per-op PSUM access penalty), split DVE/ACT by row for balance.
  - deg via quake-rsqrt on DVE; CS in gathered [u, j] layout from an fp8
    copy of Ahat+mask (exact for 0/1 data, smallest possible first DMA).
  - Only one activation-table set (Tanh/Relu/Copy), warmed by a dummy tanh.
  - x = 0.5 z z^T as bf16 hi/lo Karatsuba (error ~2e-5).
"""

import numpy as np

N = 256
DIN = 128
H1 = 256
H2 = 128
NCORES = 8
NPC = N // NCORES   # 32 rows per core
NSPT = 24           # rows using the spT accumulation path; rest are direct

# blob8 (fp8e4) column layout: CS-chain inputs + masks
_8AH = 0       # ahat two v-blocks        [128, 512]
_8MC = 512     # mc two v-blocks          [128, 64]
_8COLS = 576

# blobb (bf16) column layout (roughly in order of first use)
_ZTH = 0       # z^T hi                   [128, 256]
_ZRM = 256     # z row-major two i-blocks [128, 256]
_W1A = 512     # W1[0:128]                [128, 256]
_OCB = 768     # one-hot bf16             [128, 64]
_W2B = 832     # W2 row-blocks            [128, 256]
_W2T = 1088    # W2^T                     [128, 256]
_AH = 1344     # ahat two v-blocks        [128, 512]
_ZTL = 1856    # z^T lo                   [128, 256]
_IDB = 2112    # identity                 [128, 128]
_BCOLS = 2240

# blobf (fp32) column layout
_OCF = 0       # one-hot fp32             [128, 64]
_W1B = 64      # W1[128] as [128, 2]      [128, 2]
_FCOLS = 66

_QMAGIC = 0x5F3759DF

# rows >= EVAC_SPLIT evacuate on ACT; earlier rows on DVE (pairs below 16)
EVAC_SPLIT = 22
# software-pipeline depth (units = row-pairs for j<16, rows for j>=16)
DEPTH = 4

_PROGRAM = None
LAST_RESULTS = None
TRACE = False
TRACE_KW = {}


def _build_program():
    import concourse.bacc as bacc
    import concourse.mybir as mybir
    from concourse import tile

    F32 = mybir.dt.float32
    BF16 = mybir.dt.bfloat16
    FP8 = mybir.dt.float8e4
    I32 = mybir.dt.int32
    AF = mybir.ActivationFunctionType
    ALU = mybir.AluOpType
    DR = mybir.MatmulPerfMode.DoubleRow

    nc = bacc.Bacc()

    blob8_d = nc.dram_tensor("blob8", [128, _8COLS], FP8, kind="ExternalInput")
    blobb_d = nc.dram_tensor("blobb", [128, _BCOLS], BF16, kind="ExternalInput")
    blobf_d = nc.dram_tensor("blobf", [128, _FCOLS], F32, kind="ExternalInput")
    outc_d = nc.dram_tensor("outc", [N, NPC], F32, kind="ExternalOutput")
    x_d = nc.dram_tensor("xout", [N, N], F32, kind="ExternalOutput")
    tq_d = nc.dram_tensor("tqout", [1, NPC], F32, kind="ExternalOutput")

    with tile.TileContext(nc) as tc, tc.tile_pool(name="persist", bufs=1) as P:
        blob8 = P.tile([128, _8COLS], FP8, tag="blob8", name="blob8")
        blobb = P.tile([128, _BCOLS], BF16, tag="blobb", name="blobb")
        blobf = P.tile([128, _FCOLS], F32, tag="blobf", name="blobf")
        nc.sync.dma_start(blob8[:], blob8_d[:])          # critical path, first
        nc.scalar.dma_start(blobb[:, 0:_W2B], blobb_d[:, 0:_W2B])
        nc.scalar.dma_start(blobb[:, _W2B:_AH], blobb_d[:, _W2B:_AH])
        nc.sync.dma_start(blobb[:, _AH:_BCOLS], blobb_d[:, _AH:_BCOLS])
        nc.scalar.dma_start(blobf[:], blobf_d[:])

        ah8 = [blob8[:, _8AH + b * 256:_8AH + (b + 1) * 256] for b in range(2)]
        mc8 = [blob8[:, _8MC + b * 32:_8MC + (b + 1) * 32] for b in range(2)]

        def ahb(vb, ub=None):
            if ub is None:
                return blobb[:, _AH + vb * 256:_AH + (vb + 1) * 256]
            return blobb[:, _AH + vb * 256 + ub * 128:_AH + vb * 256 + (ub + 1) * 128]

        zthi = blobb[:, _ZTH:_ZTH + 256]
        ztlo = blobb[:, _ZTL:_ZTL + 256]
        zrm = [blobb[:, _ZRM + b * 128:_ZRM + (b + 1) * 128] for b in range(2)]
        w1a = blobb[:, _W1A:_W1A + 256]
        w2b = [blobb[:, _W2B + b * 128:_W2B + (b + 1) * 128] for b in range(2)]
        w2t = blobb[:, _W2T:_W2T + 256]
        identb = blobb[:, _IDB:_IDB + 128]
        ocb = [blobb[:, _OCB + b * 32:_OCB + (b + 1) * 32] for b in range(2)]
        ocf = [blobf[:, _OCF + b * 32:_OCF + (b + 1) * 32] for b in range(2)]
        w1bcol = [blobf[:, _W1B + b:_W1B + b + 1] for b in range(2)]

        # ---- tiny constants; dummy tanh loads the one activation table ----
        zed = P.tile([128, 1], F32, tag="zed", name="zed")
        nc.vector.memset(zed[:], 0.0)
        thd = P.tile([128, 1], F32, tag="thd", name="thd")
        nc.scalar.activation(thd[:], zed[:], AF.Tanh)
        ones_col = P.tile([128, 1], BF16, tag="ones_col", name="ones_col")
        nc.vector.memset(ones_col[:], 1.0)
        kmagic = P.tile([128, 2, 32], I32, tag="kmagic", name="kmagic")
        nc.vector.memset(kmagic[:], _QMAGIC)
        # zero-padded stage-B weights: rcpad[:, hb, j, :] has col j = rc_j[hb]
        rcpad = P.tile([128, 2, 32, 32], FP8, tag="rcpad", name="rcpad")
        nc.gpsimd.memset(rcpad[:].bitcast(I32), 0)

        # persistent sbuf intermediates
        cmax = P.tile([128, 2, 32], F32, tag="cmax", name="cmax")
        shi = P.tile([128, 2, 32], I32, tag="shi", name="shi")
        yq = P.tile([128, 2, 32], F32, tag="yq", name="yq")
        qa = P.tile([128, 2, 32], F32, tag="qa", name="qa")
        qb = P.tile([128, 2, 32], F32, tag="qb", name="qb")
        yq2 = P.tile([128, 2, 32], F32, tag="yq2", name="yq2")
        yq3 = P.tile([128, 2, 32], F32, tag="yq3", name="yq3")
        degc = P.tile([128, 2, 32], F32, tag="degc", name="degc")
        ddct = P.tile([128, 2, 32], F32, tag="ddct", name="ddct")
        z1b = P.tile([128, 2, 256], BF16, tag="z1b", name="z1b")
        zcb = P.tile([128, 32], BF16, tag="zcb", name="zcb")
        rbcb = P.tile([128, 2, 32], BF16, tag="rbcb", name="rbcb")
        qcb = P.tile([128, 32], BF16, tag="qcb", name="qcb")
        rcc = P.tile([128, 2, 32], BF16, tag="rcc", name="rcc")
        sqc = P.tile([128, 32], BF16, tag="sqc", name="sqc")
        qqsb = P.tile([1, 32], F32, tag="qqsb", name="qqsb")
        tqrow = P.tile([1, 32], F32, tag="tqrow", name="tqrow")
        xsb = P.tile([128, 2, 256], F32, tag="xsb", name="xsb")

        with tc.tile_pool(name="pre_big", bufs=2, space="PSUM") as PB, \
             tc.tile_pool(name="pre_small", bufs=3, space="PSUM") as PS:
            # ---- CS chain (critical path): CS[i_j, u] in [u, j] layout ----
            csps = PS.tile([128, 2, 32], F32, tag="pps", name="csps")
            for ub in range(2):
                for vb in range(2):
                    nc.tensor.matmul(csps[:, ub, :],
                                     ah8[vb][:, ub * 128:(ub + 1) * 128], mc8[vb],
                                     start=(vb == 0), stop=(vb == 1))
            nc.vector.tensor_scalar_max(cmax[:], csps[:], 1.0)
            # quake rsqrt: y0 = bits(K - (bits(c) >> 1)); 2 Newton iters
            nc.vector.tensor_single_scalar(shi[:], cmax[:].bitcast(I32), 1,
                                           ALU.logical_shift_right)
            nc.vector.tensor_sub(yq[:].bitcast(I32), kmagic[:], shi[:])
            nc.vector.tensor_mul(qa[:], yq[:], yq[:])
            nc.vector.scalar_tensor_tensor(qb[:], qa[:], -0.5, cmax[:],
                                           ALU.mult, ALU.mult)
            nc.vector.scalar_tensor_tensor(yq3[:], qb[:], 1.5, yq[:],
                                           ALU.add, ALU.mult)
            for b in range(2):
                nc.vector.tensor_mul(degc[:, b, :], yq3[:, b, :], mc8[b])
            nc.vector.tensor_mul(ddct[:], degc[:], yq3[:])

            # ---- Z1 = z @ W1a (stage-A weights) ----
            z1ps = PB.tile([128, 2, 256], F32, tag="ppb", name="z1ps")
            for b in range(2):
                nc.tensor.matmul(z1ps[:, b, :], zthi[:, b * 128:(b + 1) * 128],
                                 w1a, start=True, stop=True)
            nc.scalar.activation(z1b[:], z1ps[:], AF.Copy)


        # ---------------- per-row loop (software-pipelined) ----------------
        spTb = P.tile([32, 256], BF16, tag="spTb", name="spTb")
        sprime = P.tile([128, 2, 32], BF16, tag="sprime", name="sprime")
        wm = P.tile([128, 2, 32], F32, tag="wm", name="wm")
        th = P.tile([128, 2, 32], F32, tag="th", name="th")

        with tc.tile_pool(name="spt", bufs=1, space="PSUM") as SPP, \
             tc.tile_pool(name="tail_ps", bufs=1, space="PSUM") as TP, \
             tc.tile_pool(name="loop_ps2", bufs=2, space="PSUM") as LP2, \
             tc.tile_pool(name="loop_psb", bufs=2, space="PSUM") as LPB, \
             tc.tile_pool(name="loop_s", bufs=16) as LS, \
             tc.tile_pool(name="loop_f", bufs=8) as LF:
            spT = SPP.tile([128, 512], F32, tag="spt", name="spT")   # own bank
            spdir = TP.tile([128, 2, 8], F32, tag="spd", name="spdir", bufs=1)

            def qstage_zc():
                zcps = TP.tile([128, 2, 32], F32, tag="spd", name="zcps", bufs=1)
                for ib in range(2):
                    nc.tensor.matmul(zcps[:, 0, :], zrm[ib], ocb[ib],
                                     start=(ib == 0), stop=(ib == 1))
                nc.scalar.activation(zcb[:], zcps[:, 0, :], AF.Copy)

            def qstage_rbc():
                rbcps = TP.tile([128, 2, 32], F32, tag="spd", name="rbcps", bufs=1)
                for hb in range(2):
                    nc.tensor.matmul(rbcps[:, hb, :],
                                     w1a[:, hb * 128:(hb + 1) * 128],
                                     zcb[:], start=True, stop=True)
                for hb in range(2):
                    nc.scalar.activation(rbcb[:, hb, :], rbcps[:, hb, :], AF.Relu,
                                         bias=w1bcol[hb])

            def qstage_qc():
                qcps = TP.tile([128, 2, 32], F32, tag="spd", name="qcps", bufs=1)
                for hb in range(2):
                    nc.tensor.matmul(qcps[:, 0, :], w2b[hb], rbcb[:, hb, :],
                                     start=(hb == 0), stop=(hb == 1))
                nc.scalar.activation(qcb[:], qcps[:, 0, :], AF.Copy)

            def qstage_rc():
                rcps = TP.tile([128, 2, 32], F32, tag="spd", name="rcps", bufs=1)
                for hb in range(2):
                    nc.tensor.matmul(rcps[:, hb, :],
                                     w2t[:, hb * 128:(hb + 1) * 128],
                                     qcb[:], start=True, stop=True)
                nc.scalar.activation(rcc[:], rcps[:], AF.Copy)
                for hb in range(2):
                    dst = rcpad[:, hb].rearrange("p a b -> p (a b)")[:, ::33]
                    nc.vector.tensor_copy(dst, rcps[:, hb, :])

            def qstage_qq():
                nc.vector.tensor_mul(sqc[:], qcb[:], qcb[:])
                qqps = TP.tile([128, 2, 32], F32, tag="spd", name="qqps", bufs=1)
                nc.tensor.matmul(qqps[0:1, 0, :], ones_col[:], sqc[:],
                                 start=True, stop=True)
                nc.vector.tensor_copy(qqsb[:], qqps[0:1, 0, :])
                nc.scalar.activation(tqrow[:], qqsb[:], AF.Tanh)
                nc.sync.dma_start(tq_d[:], tqrow[:])

            def qstage_x():
                xps = LPB.tile([128, 2, 2, 256], F32, tag="psx", name="xps")
                for b in range(2):
                    hi_b = zthi[:, b * 128:(b + 1) * 128]
                    lo_b = ztlo[:, b * 128:(b + 1) * 128]
                    nc.tensor.matmul(xps[:, 0, b, :], hi_b, zthi,
                                     start=True, stop=False)
                    nc.tensor.matmul(xps[:, 0, b, :], hi_b, ztlo,
                                     start=False, stop=False)
                    nc.tensor.matmul(xps[:, 0, b, :], lo_b, zthi,
                                     start=False, stop=True)
                nc.scalar.activation(xsb[:], xps[:, 0], AF.Copy, scale=0.5)
                for b in range(2):
                    nc.sync.dma_start(x_d[b * 128:(b + 1) * 128, :], xsb[:, b, :])

            qstages = [qstage_zc, qstage_rbc, qstage_qc, qstage_rc, qstage_qq,
                       qstage_x]
            # trps/wps recycle spT's bank: each is first used only after the
            # previous one's last read (copy -> transpose -> fold)
            trps = SPP.tile([128, 2, 32], BF16, tag="spt", name="trps", bufs=1)
            wps = SPP.tile([128, 2, 32], F32, tag="spt", name="wps", bufs=1)

            def s_scale(j, vb, engine):
                mj = 8 * (j + 1)
                sz = min(mj, 128) if vb == 0 else mj - 128
                s = LS.tile([128, 256], BF16, tag=f"s{vb}", name=f"s{vb}_{j}")
                engine.tensor_scalar_mul(s[0:sz, 0:mj], ahb(vb)[0:sz, 0:mj],
                                         degc[0:sz, vb, j:j + 1])
                return s, sz

            def stage_a(j, psx_slice):
                mj = 8 * (j + 1)
                nvb = 1 if mj <= 128 else 2
                svt = []
                for vb in range(nvb):
                    pool = (vb == 1 and j < 28) or (vb == 0 and j < 8)
                    svt.append(s_scale(j, vb, nc.gpsimd if pool else nc.vector))
                for hb in range(2):
                    for vb in range(nvb):
                        s, sz = svt[vb]
                        nc.tensor.matmul(
                            psx_slice[:, hb, 0:mj],
                            z1b[0:sz, vb, hb * 128:(hb + 1) * 128],
                            s[0:sz, 0:mj],
                            start=(vb == 0), stop=(vb == nvb - 1))

            def stage_b(j, ft):
                # ft: [128, 2, mj] fp8 view (interleaved K-pair layout for DR)
                mj = 8 * (j + 1)
                if j < NSPT:
                    nc.tensor.matmul(spT[0:32, 0:mj], rcpad[:, :, j, :], ft,
                                     perf_mode=DR,
                                     start=(j == 0), stop=(j == NSPT - 1))
                else:
                    for vb in range(2):
                        sz = min(mj, 128) if vb == 0 else mj - 128
                        for hb in range(2):
                            nc.tensor.matmul(
                                spdir[0:sz, vb, j - NSPT:j - NSPT + 1],
                                ft[:, hb, vb * 128:vb * 128 + sz],
                                rcc[:, hb, j:j + 1],
                                start=(hb == 0), stop=(hb == 1))

            def tail_a():
                # spT closed: copy/transpose/scale the first NSPT columns
                nc.vector.tensor_copy(spTb[:], spT[0:32, 0:256])
                for ub in range(2):
                    nc.tensor.transpose(trps[:, ub, :],
                                        spTb[:, ub * 128:(ub + 1) * 128],
                                        identb[0:32, 0:32])
                nc.vector.tensor_mul(sprime[:], trps[:], ddct[:])

            # pipeline: emit stage_b DEPTH units behind stage_a (ascending
            # rows; 0..23 accumulate into spT, 24..31 use direct matvecs so
            # the spT transpose tail overlaps the last rows)
            pend = []   # (j, ft_view)

            def flush(limit):
                while len(pend) > limit:
                    jj, ftv = pend.pop(0)
                    stage_b(jj, ftv)
                    if jj == NSPT - 1:
                        tail_a()

            for jp in range(8):          # rows 0..15 in pairs (1 bank)
                j0, j1 = 2 * jp, 2 * jp + 1
                mj1 = 8 * (j1 + 1)
                psX2 = LP2.tile([128, 2, 2, 128], F32, tag="psx2",
                                name=f"psx2_{jp}")
                stage_a(j0, psX2[:, 0])
                stage_a(j1, psX2[:, 1])
                ft2 = LF.tile([128, 2, 2, 128], FP8, tag="ft2", name=f"ftp{jp}",
                              bufs=8)
                if jp >= 2:
                    nc.scalar.activation(ft2[:, :, :, 0:mj1],
                                         psX2[:, :, :, 0:mj1], AF.Relu)
                else:
                    nc.vector.tensor_scalar_max(ft2[:, :, :, 0:mj1],
                                                psX2[:, :, :, 0:mj1], 0.0)
                pend.append((j0, ft2[:, 0, :, 0:8 * (j0 + 1)]))
                pend.append((j1, ft2[:, 1, :, 0:mj1]))
                if jp < len(qstages):
                    qstages[jp]()

            for jp in range(8, 16):      # rows 16..31 in pairs (2 banks)
                j0, j1 = 2 * jp, 2 * jp + 1
                mj1 = 8 * (j1 + 1)
                psXB = LPB.tile([128, 2, 2, 256], F32, tag="psx", name=f"psxb{jp}")
                stage_a(j0, psXB[:, 0])
                stage_a(j1, psXB[:, 1])
                ftb = LF.tile([128, 2, 2, 256], FP8, tag="ftb", name=f"ftb{jp}")
                if jp == 15:
                    nc.scalar.activation(ftb[:, 0, :, 0:8 * (j0 + 1)],
                                         psXB[:, 0, :, 0:8 * (j0 + 1)], AF.Relu)
                    nc.vector.tensor_scalar_max(ftb[:, 1, :, 0:mj1],
                                                psXB[:, 1, :, 0:mj1], 0.0)
                elif jp in (8, 9, 10, 13):
                    nc.vector.tensor_scalar_max(ftb[:, :, :, 0:mj1],
                                                psXB[:, :, :, 0:mj1], 0.0)
                else:
                    nc.scalar.activation(ftb[:, :, :, 0:mj1],
                                         psXB[:, :, :, 0:mj1], AF.Relu)
                pend.append((j0, ftb[:, 0, :, 0:8 * (j0 + 1)]))
                pend.append((j1, ftb[:, 1, :, 0:mj1]))
                flush(max(DEPTH, 14 - 2 * (jp - 8)))
            flush(0)

            # ---- tail B, part 1: columns 0:NSPT (spT rows), ships early ----
            for ub in range(2):
                for vb in range(2):
                    nc.tensor.matmul(wps[:, ub, 0:NSPT], ahb(vb, ub),
                                     sprime[:, vb, 0:NSPT],
                                     start=(vb == 0), stop=(vb == 1))
            nc.vector.tensor_mul(wm[:, :, 0:NSPT], wps[:, :, 0:NSPT],
                                 degc[:, :, 0:NSPT])
            nc.scalar.activation(th[:, :, 0:NSPT], wm[:, :, 0:NSPT], AF.Tanh)
            nc.scalar.dma_start(outc_d[0:128, 0:NSPT], th[:, 0, 0:NSPT])
            nc.sync.dma_start(outc_d[128:256, 0:NSPT], th[:, 1, 0:NSPT])

            # ---- tail B, part 2: columns NSPT:NPC (direct rows) ----
            ND = NPC - NSPT
            nc.vector.tensor_mul(sprime[:, :, NSPT:NPC], spdir[:],
                                 ddct[:, :, NSPT:NPC])
            wps2 = TP.tile([128, 2, 8], F32, tag="spd", name="wps2", bufs=1)
            for ub in range(2):
                for vb in range(2):
                    nc.tensor.matmul(wps2[:, ub, :], ahb(vb, ub),
                                     sprime[:, vb, NSPT:NPC],
                                     start=(vb == 0), stop=(vb == 1))
            nc.vector.tensor_mul(wm[:, :, NSPT:NPC], wps2[:], degc[:, :, NSPT:NPC])
            nc.scalar.activation(th[:, :, NSPT:NPC], wm[:, :, NSPT:NPC], AF.Tanh)
            nc.scalar.dma_start(outc_d[0:128, NSPT:NPC], th[:, 0, NSPT:NPC])
            nc.sync.dma_start(outc_d[128:256, NSPT:NPC], th[:, 1, NSPT:NPC])

    nc.finalize()
    return nc


def _get_program():
    global _PROGRAM
    if _PROGRAM is None:
        _PROGRAM = _build_program()
    return _PROGRAM


def _pack_inputs(z, adj, W1, W2):
    import ml_dtypes
    bf = ml_dtypes.bfloat16
    f8 = ml_dtypes.float8_e4m3fn

    idx = np.arange(N)
    ahat = np.maximum(adj, np.eye(N, dtype=np.float32))
    zt = z.T.astype(np.float32)                     # [128, 256]
    zthi = zt.astype(bf)
    ztlo = (zt - zthi.astype(np.float32)).astype(bf)

    base_8 = np.zeros((128, _8COLS), f8)
    base_8[:, _8AH:_8AH + 256] = ahat[0:128].astype(f8)
    base_8[:, _8AH + 256:_8AH + 512] = ahat[128:256].astype(f8)

    base_b = np.zeros((128, _BCOLS), bf)
    base_b[:, _AH:_AH + 256] = ahat[0:128].astype(bf)
    base_b[:, _AH + 256:_AH + 512] = ahat[128:256].astype(bf)
    base_b[:, _ZTH:_ZTH + 256] = zthi
    base_b[:, _ZTL:_ZTL + 256] = ztlo
    base_b[:, _ZRM:_ZRM + 128] = z[0:128].astype(bf)
    base_b[:, _ZRM + 128:_ZRM + 256] = z[128:256].astype(bf)
    base_b[:, _W1A:_W1A + 256] = W1[0:128].astype(bf)
    base_b[:, _W2B:_W2B + 128] = W2[0:128].astype(bf)
    base_b[:, _W2B + 128:_W2B + 256] = W2[128:256].astype(bf)
    base_b[:, _W2T:_W2T + 256] = W2.T.astype(bf)
    base_b[:, _IDB:_IDB + 128] = np.eye(128, dtype=np.float32).astype(bf)

    base_f = np.zeros((128, _FCOLS), np.float32)
    base_f[:, _W1B] = W1[128, 0:128]
    base_f[:, _W1B + 1] = W1[128, 128:256]

    in_maps = []
    for c in range(NCORES):
        ii = np.arange(c, N, NCORES)
        OC = np.zeros((N, NPC), np.float32)
        OC[ii, np.arange(NPC)] = 1.0
        MC = (idx[:, None] < ii[None, :]).astype(np.float32)
        b8 = base_8.copy()
        b8[:, _8MC:_8MC + 32] = MC[0:128].astype(f8)
        b8[:, _8MC + 32:_8MC + 64] = MC[128:256].astype(f8)
        bb = base_b.copy()
        bb[:, _OCB:_OCB + 32] = OC[0:128].astype(bf)
        bb[:, _OCB + 32:_OCB + 64] = OC[128:256].astype(bf)
        bfl = base_f.copy()
        bfl[:, _OCF:_OCF + 32] = OC[0:128]
        bfl[:, _OCF + 32:_OCF + 64] = OC[128:256]
        in_maps.append({"blob8": b8, "blobb": bb, "blobf": bfl})
    return in_maps


def kernel(z, adj, W1, W2):
    global LAST_RESULTS
    from concourse.bass_utils import run_bass_kernel_spmd

    z = np.ascontiguousarray(np.asarray(z, np.float32))
    adj = np.ascontiguousarray(np.asarray(adj, np.float32))
    W1 = np.ascontiguousarray(np.asarray(W1, np.float32))
    W2 = np.ascontiguousarray(np.asarray(W2, np.float32))

    nc = _get_program()
    in_maps = _pack_inputs(z, adj, W1, W2)
    res = run_bass_kernel_spmd(nc, in_maps, list(range(NCORES)),
                               trace=TRACE, **TRACE_KW)
    LAST_RESULTS = res
    supp = np.zeros((N, N), np.float32)
    for c in range(NCORES):
        ii = np.arange(c, N, NCORES)
        supp[ii, :] = 0.5 * res.results[c]["outc"].T
        supp[ii, ii] += 0.5 * res.results[c]["tqout"][0]
    x = res.results[0]["xout"]
    return (x + supp + supp.T).astype(np.float32)
